# revision 11
# baseline (speedup 1.0000x reference)
"""Trainium2 Bass kernel for nn_Conv2D_80796924772741.

Depthwise (grouped, F=64) 3x3 valid conv over [F, 514, 514, 4] int8 with
per-channel int8 weights + int32 bias, followed by exact fixed-point requant
  acc = conv + b ;  res = (acc*1245 + 2^17) >> 18
  out = clip(res - 5, -128, 127) int8

Sharding: F=64 split across 8 NeuronCores (8 channels each), embarrassingly
parallel.

Per-core compute:
 - PE: per channel, conv via Toeplitz-band stationary matmuls over H-windows
   (contraction = 128 rows: 2 ones rows carrying the int32 bias b split as
   8*floor(b/8) + (b mod 8), then 126 data rows; 3 matmuls for the 3 W-taps,
   W-shift = +4n free-dim offset since (w,d) is flattened). PSUM [124, 2048]
   spans 4 banks; 4 chunks x 3 taps accumulate per window, so PSUM = acc.
 - Requant in 3 elementwise ops, exploiting the HW's RNE+saturate conversion
   on every fp32->int write (verified on-device). With 1245 = 5*256 - 35 and
   K0 = 2^17 - 5*2^18 (folds the rounding bias and the -5 zero point):
     E   = floor((K0 - 35*acc)/256)       [ACT: acc*(-35/256) - 4608.498046875
                                           -> int16; RNE w/ -255/512 centering
                                           == exact floor]
     S   = 5*acc + E                      [DVE scalar_tensor_tensor from PSUM
                                           -> int32, exact in fp32]
     out = clip(floor(S/1024), -128, 127) [S*2^-10 - 0.49951171875 -> int8;
                                           RNE+saturate == floor + clip]
   Every intermediate is exactly representable in fp32; the chain was
   verified bit-exact against the int64 reference over the full acc range.
   The final op alternates DVE/ACT per window to balance engine load.
 - Output rows 496..511: two block-diagonal matmuls (4 channels each,
   74 contraction rows -> 64 outputs) reuse the same requant.
"""

import numpy as np
import ml_dtypes

F_PER_CORE = 8
H_IN = 514
W_IN = 514
D = 4
H_OUT = 512
WD_OUT = 2048  # 512 * 4
FREE_IN = W_IN * D  # 2056
N_CHUNK = 512
N_CORES = 8

M_R = 124                      # output rows per full window
N_WIN = 4                      # full windows: rows 0..495
M_LEFT = 16                    # leftover rows per channel (496..511)
K_LEFT = M_LEFT + 2            # 18 data rows per channel in leftover MM
CH_HALF = 4                    # channels per leftover MM

SCALE_E = -0.13671875          # -35/256
BIAS_A = -4608.498046875       # (2*K0 - 255)/512, K0 = 2^17 - 5*2^18
SCALE_S = 0.0009765625         # 2^-10
BIAS_D = 0.49951171875         # 0.5 - 2^-11 (subtracted)


def _bias_rows(b: int):
    bh = int(b) >> 3
    return float(8 * bh), float(int(b) - 8 * bh)


def _build_lhsT(w_core: np.ndarray, b_core: np.ndarray) -> np.ndarray:
    """[128, 8*3*124] bf16 stationary: per (channel, w-tap) a Toeplitz band.

    Column block (f*3 + n)*124 : +124 holds T_n for channel f:
    T_n[2 + i + m, i] = w[f, m, n]  (rows 2..127 are conv data partitions).
    T_0[0, i] = 8*floor(b/8) ; T_0[1, i] = b mod 8  (bias rows, multiplied by
    all-ones rhs partitions 0/1).
    """
    out = np.zeros((128, F_PER_CORE * 3 * M_R), dtype=np.float32)
    idx = np.arange(M_R)
    for f in range(F_PER_CORE):
        bh8, bl = _bias_rows(int(b_core[f]))
        for n in range(3):
            base = (f * 3 + n) * M_R
            if n == 0:
                out[0, base : base + M_R] = bh8
                out[1, base : base + M_R] = bl
            for m in range(3):
                out[2 + idx + m, base + idx] = float(int(w_core[f, m, n, 0]))
    return out.astype(ml_dtypes.bfloat16)


def _build_lhsT_left(w_core: np.ndarray, b_core: np.ndarray, half: int) -> np.ndarray:
    """[74, 3*64] bf16 block-diagonal stationary for leftover rows 496..511.

    Channels 4*half..4*half+3. Tap-n block at cols n*64:
    T[2 + 18*j + i + m, n*64 + 16*j + i] = w[f, m, n]  (j = f - 4*half, i<16)
    plus bias rows 0/1 on the n==0 block.
    """
    out = np.zeros((2 + CH_HALF * K_LEFT, 3 * CH_HALF * M_LEFT), dtype=np.float32)
    idx = np.arange(M_LEFT)
    for j in range(CH_HALF):
        f = CH_HALF * half + j
        bh8, bl = _bias_rows(int(b_core[f]))
        for n in range(3):
            base = n * CH_HALF * M_LEFT + j * M_LEFT
            if n == 0:
                out[0, base : base + M_LEFT] = bh8
                out[1, base : base + M_LEFT] = bl
            for m in range(3):
                out[2 + j * K_LEFT + idx + m, base + idx] = float(
                    int(w_core[f, m, n, 0])
                )
    return out.astype(ml_dtypes.bfloat16)


_PROGRAM_CACHE = {}


def _build_program():
    import concourse.bass as bass
    import concourse.tile as tile
    from concourse import bacc, mybir

    nc = bacc.Bacc(
        "TRN2", target_bir_lowering=False, debug=False, num_devices=N_CORES
    )
    dt = mybir.dt
    Alu = mybir.AluOpType
    Act = mybir.ActivationFunctionType

    x_d = nc.dram_tensor(
        "x", [F_PER_CORE, H_IN, FREE_IN], dt.int8, kind="ExternalInput"
    ).ap()
    lhsT_d = nc.dram_tensor(
        "lhsT", [128, F_PER_CORE * 3 * M_R], dt.bfloat16, kind="ExternalInput"
    ).ap()
    lhsTl_d = nc.dram_tensor(
        "lhsTl", [2, 2 + CH_HALF * K_LEFT, 3 * CH_HALF * M_LEFT], dt.bfloat16,
        kind="ExternalInput",
    ).ap()
    ones_d = nc.dram_tensor("ones2", [2, FREE_IN], dt.bfloat16, kind="ExternalInput").ap()
    y_d = nc.dram_tensor(
        "y", [F_PER_CORE, H_OUT, WD_OUT], dt.int8, kind="ExternalOutput"
    ).ap()

    KL = 2 + CH_HALF * K_LEFT  # 74

    with tile.TileContext(nc) as tc:
        with (
            tc.tile_pool(name="const", bufs=1) as const_pool,
            tc.tile_pool(name="xin", bufs=3) as x_pool,
            tc.tile_pool(name="psum", bufs=4, space="PSUM") as psum_pool,
            tc.tile_pool(name="etile", bufs=3) as e_pool,
            tc.tile_pool(name="stile", bufs=3) as s_pool,
            tc.tile_pool(name="otile", bufs=3) as o_pool,
        ):
            lhsT_t = const_pool.tile([128, F_PER_CORE * 3 * M_R], dt.bfloat16)
            nc.sync.dma_start(lhsT_t[:], lhsT_d[:])
            lhsTl_t = const_pool.tile([KL, 2 * 3 * CH_HALF * M_LEFT], dt.bfloat16)
            nc.sync.dma_start(
                lhsTl_t[:, 0 : 3 * CH_HALF * M_LEFT], lhsTl_d[0, :, :]
            )
            nc.sync.dma_start(
                lhsTl_t[:, 3 * CH_HALF * M_LEFT :], lhsTl_d[1, :, :]
            )

            GRP = 1024  # requant group width: 2 PSUM banks

            gi = [0]

            def requant(ps_ap, rows):
                """PSUM acc -> E16 (ACT) -> S fp32 (DVE stt) -> out8."""
                et = e_pool.tile([M_R, GRP], dt.int16)
                nc.scalar.activation(
                    et[0:rows, :], ps_ap, Act.Copy, bias=BIAS_A, scale=SCALE_E
                )
                st = s_pool.tile([M_R, GRP], dt.float32)
                nc.vector.scalar_tensor_tensor(
                    st[0:rows, :], ps_ap, 5.0, et[0:rows, :], Alu.mult, Alu.add
                )
                ot = o_pool.tile([M_R, GRP], dt.int8)
                # rotate the final op across GPSIMD/ACT/DVE (weighted)
                k = gi[0] % 20
                gi[0] += 1
                if k % 2 == 0:  # 50% gpsimd
                    nc.gpsimd.tensor_scalar(
                        ot[0:rows, :], st[0:rows, :], SCALE_S, BIAS_D,
                        Alu.mult, Alu.subtract,
                    )
                elif k in (1, 5, 9, 13, 17):  # 25% act
                    nc.scalar.activation(
                        ot[0:rows, :], st[0:rows, :], Act.Copy,
                        bias=-BIAS_D, scale=SCALE_S,
                    )
                else:  # 25% dve
                    nc.vector.tensor_scalar(
                        ot[0:rows, :], st[0:rows, :], SCALE_S, BIAS_D,
                        Alu.mult, Alu.subtract,
                    )
                return ot

            def conv_and_requant(lhs_t, lhs_base, xt_ap, rows, y_slicer):
                """One window: 2 chunk-pair groups of (6 MMs -> requant -> DMA)."""
                for p in range(2):
                    ps = psum_pool.tile([M_R, GRP], dt.float32, tag="ps")
                    for n in range(3):
                        base = lhs_base(n)
                        for c in (2 * p, 2 * p + 1):
                            nc.tensor.matmul(
                                ps[0:rows, (c - 2 * p) * N_CHUNK : (c - 2 * p + 1) * N_CHUNK],
                                lhs_t[:, base : base + rows],
                                xt_ap[:, c * N_CHUNK + 4 * n : c * N_CHUNK + 4 * n + N_CHUNK],
                                start=(n == 0),
                                stop=(n == 2),
                                skip_group_check=True,
                            )
                    ot = requant(ps[0:rows, :], rows)
                    nc.sync.dma_start(y_slicer(p), ot[0:rows, :])

            for f in range(F_PER_CORE):
                for w in range(N_WIN):
                    r0 = w * M_R
                    xt = x_pool.tile([128, FREE_IN], dt.bfloat16, tag="xt")
                    nc.sync.dma_start(xt[0:2, :], ones_d[:])
                    nc.gpsimd.dma_start(xt[2:128, :], x_d[f, r0 : r0 + M_R + 2, :])
                    conv_and_requant(
                        lhsT_t,
                        lambda n, f=f: (f * 3 + n) * M_R,
                        xt[:],
                        M_R,
                        lambda p, f=f, r0=r0: y_d[
                            f, r0 : r0 + M_R, p * GRP : (p + 1) * GRP
                        ],
                    )

            # leftover rows 496..511: two block-diag MMs of 4 channels each
            rows_l = CH_HALF * M_LEFT  # 64
            for half in range(2):
                xl = x_pool.tile([128, FREE_IN], dt.bfloat16, tag="xt")
                nc.sync.dma_start(xl[0:2, :], ones_d[:])
                nc.gpsimd.dma_start(
                    xl[2:KL, :],
                    x_d[CH_HALF * half : CH_HALF * half + CH_HALF,
                        N_WIN * M_R : N_WIN * M_R + K_LEFT, :],
                )
                conv_and_requant(
                    lhsTl_t,
                    lambda n, half=half: half * 3 * rows_l + n * rows_l,
                    xl[0:KL, :],
                    rows_l,
                    lambda p, half=half: y_d[
                        CH_HALF * half : CH_HALF * half + CH_HALF,
                        N_WIN * M_R : H_OUT,
                        p * GRP : (p + 1) * GRP,
                    ],
                )

    nc.compile()
    return nc


def make_in_maps(x: np.ndarray, w: np.ndarray, b: np.ndarray) -> list:
    ones2 = np.ones((2, FREE_IN), dtype=np.float32).astype(ml_dtypes.bfloat16)
    in_maps = []
    for core in range(N_CORES):
        lo = core * F_PER_CORE
        hi = lo + F_PER_CORE
        x_shard = np.ascontiguousarray(x[lo:hi]).reshape(F_PER_CORE, H_IN, FREE_IN)
        lhsT = _build_lhsT(w[lo:hi], b[lo:hi])
        lhsTl = np.stack(
            [_build_lhsT_left(w[lo:hi], b[lo:hi], h) for h in range(2)]
        )
        in_maps.append(
            {"x": x_shard, "lhsT": lhsT, "lhsTl": lhsTl, "ones2": ones2}
        )
    return in_maps


def kernel(x: np.ndarray, w: np.ndarray, b: np.ndarray) -> np.ndarray:
    """x: int8 [64, 514, 514, 4]; w: int8 [64, 3, 3, 1]; b: int32 [64].

    Returns int8 [64, 512, 512, 4].
    """
    from concourse.bass_utils import run_bass_kernel_spmd

    if "nc" not in _PROGRAM_CACHE:
        _PROGRAM_CACHE["nc"] = _build_program()
    nc = _PROGRAM_CACHE["nc"]

    F = x.shape[0]
    assert F == N_CORES * F_PER_CORE

    res = run_bass_kernel_spmd(nc, make_in_maps(x, w, b), core_ids=list(range(N_CORES)))

    out = np.empty((F, H_OUT, 512, D), dtype=np.int8)
    for core in range(N_CORES):
        lo = core * F_PER_CORE
        y = res.results[core]["y"]  # [8, 512, 2048] int8
        out[lo : lo + F_PER_CORE] = y.reshape(F_PER_CORE, H_OUT, 512, D)
    return out


# revision 12
# speedup vs baseline: 3.0315x; 3.0315x over previous
"""Trainium2 Bass kernel for nn_Conv2D_80796924772741.

Depthwise (grouped, F=64) 3x3 valid conv over [F, 514, 514, 4] int8 with
per-channel int8 weights + int32 bias, followed by exact fixed-point requant
  acc = conv + b ;  res = (acc*1245 + 2^17) >> 18
  out = clip(res - 5, -128, 127) int8

Sharding: F=64 split across 8 NeuronCores (8 channels each), embarrassingly
parallel.

Per-core compute:
 - PE: per channel, conv via Toeplitz-band stationary matmuls over H-windows
   (contraction = 128 rows: 2 ones rows carrying the int32 bias b split as
   8*floor(b/8) + (b mod 8), then 126 data rows; 3 matmuls for the 3 W-taps,
   W-shift = +4n free-dim offset since (w,d) is flattened). PSUM [124, 2048]
   spans 4 banks; 4 chunks x 3 taps accumulate per window, so PSUM = acc.
 - Requant in 3 elementwise ops, exploiting the HW's RNE+saturate conversion
   on every fp32->int write (verified on-device). With 1245 = 5*256 - 35 and
   K0 = 2^17 - 5*2^18 (folds the rounding bias and the -5 zero point):
     E   = floor((K0 - 35*acc)/256)       [ACT: acc*(-35/256) - 4608.498046875
                                           -> int16; RNE w/ -255/512 centering
                                           == exact floor]
     S   = 5*acc + E                      [DVE scalar_tensor_tensor from PSUM
                                           -> int32, exact in fp32]
     out = clip(floor(S/1024), -128, 127) [S*2^-10 - 0.49951171875 -> int8;
                                           RNE+saturate == floor + clip]
   Every intermediate is exactly representable in fp32; the chain was
   verified bit-exact against the int64 reference over the full acc range.
   The final op alternates DVE/ACT per window to balance engine load.
 - Output rows 496..511: two block-diagonal matmuls (4 channels each,
   74 contraction rows -> 64 outputs) reuse the same requant.
"""

import numpy as np
import ml_dtypes

F_PER_CORE = 8
H_IN = 514
W_IN = 514
D = 4
H_OUT = 512
WD_OUT = 2048  # 512 * 4
FREE_IN = W_IN * D  # 2056
N_CHUNK = 512
N_CORES = 8

M_R = 124                      # output rows per full window
N_WIN = 4                      # full windows: rows 0..495
M_LEFT = 16                    # leftover rows per channel (496..511)
K_LEFT = M_LEFT + 2            # 18 data rows per channel in leftover MM
CH_HALF = 4                    # channels per leftover MM

SCALE_E = -0.13671875          # -35/256
BIAS_A = -4608.498046875       # (2*K0 - 255)/512, K0 = 2^17 - 5*2^18
SCALE_S = 0.0009765625         # 2^-10
BIAS_D = 0.49951171875         # 0.5 - 2^-11 (subtracted)


def _bias_rows(b: int):
    bh = int(b) >> 3
    return float(8 * bh), float(int(b) - 8 * bh)


def _build_lhsT(w_core: np.ndarray, b_core: np.ndarray) -> np.ndarray:
    """[128, 8*3*124] bf16 stationary: per (channel, w-tap) a Toeplitz band.

    Column block (f*3 + n)*124 : +124 holds T_n for channel f:
    T_n[2 + i + m, i] = w[f, m, n]  (rows 2..127 are conv data partitions).
    T_0[0, i] = 8*floor(b/8) ; T_0[1, i] = b mod 8  (bias rows, multiplied by
    all-ones rhs partitions 0/1).
    """
    out = np.zeros((128, F_PER_CORE * 3 * M_R), dtype=np.float32)
    idx = np.arange(M_R)
    for f in range(F_PER_CORE):
        bh8, bl = _bias_rows(int(b_core[f]))
        for n in range(3):
            base = (f * 3 + n) * M_R
            if n == 0:
                out[0, base : base + M_R] = bh8
                out[1, base : base + M_R] = bl
            for m in range(3):
                out[2 + idx + m, base + idx] = float(int(w_core[f, m, n, 0]))
    return out.astype(ml_dtypes.bfloat16)


def _build_lhsT_left(w_core: np.ndarray, b_core: np.ndarray, half: int) -> np.ndarray:
    """[74, 3*64] bf16 block-diagonal stationary for leftover rows 496..511.

    Channels 4*half..4*half+3. Tap-n block at cols n*64:
    T[2 + 18*j + i + m, n*64 + 16*j + i] = w[f, m, n]  (j = f - 4*half, i<16)
    plus bias rows 0/1 on the n==0 block.
    """
    out = np.zeros((2 + CH_HALF * K_LEFT, 3 * CH_HALF * M_LEFT), dtype=np.float32)
    idx = np.arange(M_LEFT)
    for j in range(CH_HALF):
        f = CH_HALF * half + j
        bh8, bl = _bias_rows(int(b_core[f]))
        for n in range(3):
            base = n * CH_HALF * M_LEFT + j * M_LEFT
            if n == 0:
                out[0, base : base + M_LEFT] = bh8
                out[1, base : base + M_LEFT] = bl
            for m in range(3):
                out[2 + j * K_LEFT + idx + m, base + idx] = float(
                    int(w_core[f, m, n, 0])
                )
    return out.astype(ml_dtypes.bfloat16)


_PROGRAM_CACHE = {}


def _build_program():
    import concourse.bass as bass
    import concourse.tile as tile
    from concourse import bacc, mybir

    nc = bacc.Bacc(
        "TRN2", target_bir_lowering=False, debug=False, num_devices=N_CORES
    )
    dt = mybir.dt
    Alu = mybir.AluOpType
    Act = mybir.ActivationFunctionType

    x_d = nc.dram_tensor(
        "x", [F_PER_CORE, H_IN, FREE_IN], dt.int8, kind="ExternalInput"
    ).ap()
    lhsT_d = nc.dram_tensor(
        "lhsT", [128, F_PER_CORE * 3 * M_R], dt.bfloat16, kind="ExternalInput"
    ).ap()
    lhsTl_d = nc.dram_tensor(
        "lhsTl", [2, 2 + CH_HALF * K_LEFT, 3 * CH_HALF * M_LEFT], dt.bfloat16,
        kind="ExternalInput",
    ).ap()
    ones_d = nc.dram_tensor("ones2", [2, FREE_IN], dt.bfloat16, kind="ExternalInput").ap()
    y_d = nc.dram_tensor(
        "y", [F_PER_CORE, H_OUT, WD_OUT], dt.int8, kind="ExternalOutput"
    ).ap()

    KL = 2 + CH_HALF * K_LEFT  # 74

    with tile.TileContext(nc) as tc:
        with (
            tc.tile_pool(name="const", bufs=1) as const_pool,
            tc.tile_pool(name="xin", bufs=3) as x_pool,
            tc.tile_pool(name="psum", bufs=4, space="PSUM") as psum_pool,
            tc.tile_pool(name="etile", bufs=3) as e_pool,
            tc.tile_pool(name="stile", bufs=3) as s_pool,
            tc.tile_pool(name="otile", bufs=3) as o_pool,
        ):
            lhsT_t = const_pool.tile([128, F_PER_CORE * 3 * M_R], dt.bfloat16)
            nc.sync.dma_start(lhsT_t[:], lhsT_d[:])
            lhsTl_t = const_pool.tile([KL, 2 * 3 * CH_HALF * M_LEFT], dt.bfloat16)
            nc.sync.dma_start(
                lhsTl_t[:, 0 : 3 * CH_HALF * M_LEFT], lhsTl_d[0, :, :]
            )
            nc.sync.dma_start(
                lhsTl_t[:, 3 * CH_HALF * M_LEFT :], lhsTl_d[1, :, :]
            )

            GRP = 1024  # requant group width: 2 PSUM banks

            gi = [0]

            def requant(ps_ap, rows):
                """PSUM acc -> E16 (ACT) -> S fp32 (DVE stt) -> out8."""
                et = e_pool.tile([M_R, GRP], dt.int16)
                nc.scalar.activation(
                    et[0:rows, :], ps_ap, Act.Copy, bias=BIAS_A, scale=SCALE_E
                )
                st = s_pool.tile([M_R, GRP], dt.float32)
                nc.vector.scalar_tensor_tensor(
                    st[0:rows, :], ps_ap, 5.0, et[0:rows, :], Alu.mult, Alu.add
                )
                ot = o_pool.tile([M_R, GRP], dt.int8)
                # alternate the final op between ACT and DVE (gpsimd tensor
                # ops measured ~6x slower than spec on int8-out: unusable)
                k = gi[0]
                gi[0] += 1
                if k % 2 == 0:
                    nc.scalar.activation(
                        ot[0:rows, :], st[0:rows, :], Act.Copy,
                        bias=-BIAS_D, scale=SCALE_S,
                    )
                else:
                    nc.vector.tensor_scalar(
                        ot[0:rows, :], st[0:rows, :], SCALE_S, BIAS_D,
                        Alu.mult, Alu.subtract,
                    )
                return ot

            def conv_and_requant(lhs_t, lhs_base, xt_ap, rows, y_slicer):
                """One window: 2 chunk-pair groups of (6 MMs -> requant -> DMA)."""
                for p in range(2):
                    ps = psum_pool.tile([M_R, GRP], dt.float32, tag="ps")
                    for n in range(3):
                        base = lhs_base(n)
                        for c in (2 * p, 2 * p + 1):
                            nc.tensor.matmul(
                                ps[0:rows, (c - 2 * p) * N_CHUNK : (c - 2 * p + 1) * N_CHUNK],
                                lhs_t[:, base : base + rows],
                                xt_ap[:, c * N_CHUNK + 4 * n : c * N_CHUNK + 4 * n + N_CHUNK],
                                start=(n == 0),
                                stop=(n == 2),
                                skip_group_check=True,
                            )
                    ot = requant(ps[0:rows, :], rows)
                    nc.sync.dma_start(y_slicer(p), ot[0:rows, :])

            for f in range(F_PER_CORE):
                for w in range(N_WIN):
                    r0 = w * M_R
                    xt = x_pool.tile([128, FREE_IN], dt.bfloat16, tag="xt")
                    nc.sync.dma_start(xt[0:2, :], ones_d[:])
                    nc.gpsimd.dma_start(xt[2:128, :], x_d[f, r0 : r0 + M_R + 2, :])
                    conv_and_requant(
                        lhsT_t,
                        lambda n, f=f: (f * 3 + n) * M_R,
                        xt[:],
                        M_R,
                        lambda p, f=f, r0=r0: y_d[
                            f, r0 : r0 + M_R, p * GRP : (p + 1) * GRP
                        ],
                    )

            # leftover rows 496..511: two block-diag MMs of 4 channels each
            rows_l = CH_HALF * M_LEFT  # 64
            for half in range(2):
                xl = x_pool.tile([128, FREE_IN], dt.bfloat16, tag="xt")
                nc.sync.dma_start(xl[0:2, :], ones_d[:])
                nc.gpsimd.dma_start(
                    xl[2:KL, :],
                    x_d[CH_HALF * half : CH_HALF * half + CH_HALF,
                        N_WIN * M_R : N_WIN * M_R + K_LEFT, :],
                )
                conv_and_requant(
                    lhsTl_t,
                    lambda n, half=half: half * 3 * rows_l + n * rows_l,
                    xl[0:KL, :],
                    rows_l,
                    lambda p, half=half: y_d[
                        CH_HALF * half : CH_HALF * half + CH_HALF,
                        N_WIN * M_R : H_OUT,
                        p * GRP : (p + 1) * GRP,
                    ],
                )

    nc.compile()
    return nc


def make_in_maps(x: np.ndarray, w: np.ndarray, b: np.ndarray) -> list:
    ones2 = np.ones((2, FREE_IN), dtype=np.float32).astype(ml_dtypes.bfloat16)
    in_maps = []
    for core in range(N_CORES):
        lo = core * F_PER_CORE
        hi = lo + F_PER_CORE
        x_shard = np.ascontiguousarray(x[lo:hi]).reshape(F_PER_CORE, H_IN, FREE_IN)
        lhsT = _build_lhsT(w[lo:hi], b[lo:hi])
        lhsTl = np.stack(
            [_build_lhsT_left(w[lo:hi], b[lo:hi], h) for h in range(2)]
        )
        in_maps.append(
            {"x": x_shard, "lhsT": lhsT, "lhsTl": lhsTl, "ones2": ones2}
        )
    return in_maps


def kernel(x: np.ndarray, w: np.ndarray, b: np.ndarray) -> np.ndarray:
    """x: int8 [64, 514, 514, 4]; w: int8 [64, 3, 3, 1]; b: int32 [64].

    Returns int8 [64, 512, 512, 4].
    """
    from concourse.bass_utils import run_bass_kernel_spmd

    if "nc" not in _PROGRAM_CACHE:
        _PROGRAM_CACHE["nc"] = _build_program()
    nc = _PROGRAM_CACHE["nc"]

    F = x.shape[0]
    assert F == N_CORES * F_PER_CORE

    res = run_bass_kernel_spmd(nc, make_in_maps(x, w, b), core_ids=list(range(N_CORES)))

    out = np.empty((F, H_OUT, 512, D), dtype=np.int8)
    for core in range(N_CORES):
        lo = core * F_PER_CORE
        y = res.results[core]["y"]  # [8, 512, 2048] int8
        out[lo : lo + F_PER_CORE] = y.reshape(F_PER_CORE, H_OUT, 512, D)
    return out
